# revision 34
# baseline (speedup 1.0000x reference)
"""Multi-head attention TRN2 kernel (8 NeuronCores).

Sharding: core (2b + h2) handles batch b (of 4) and head-half h2 (8 of 16
heads).  Each core projects its batch's Q/K/V through its 512-column slice
of Wq/Wk/Wv, runs causal flash-attention for its 8 heads, and computes a
partial output projection through its 512 rows of Wo^T.  The two partial
outputs per batch are summed on the host (the "all-reduce after W_o"),
plus the output bias.

All matmuls run in fp16 with fp32 PSUM accumulation.  Scores are computed
transposed (S^T[kj, qi] = kT.T @ qT) so the softmax sum comes for free from
a ones-column appended to V (padded to 128 columns so LDWEIGHTS gets fast
weight load), and the attention output lands f-major, which is exactly the
lhsT layout the Wo matmul needs.  Normalisation (divide by the softmax sum)
is a DVE fast-reciprocal + GPSIMD partition-broadcast + DVE multiply.

Causal handling: on the 4 diagonal key-tiles of each 512-token query block
only the q-columns at or past the causal boundary are computed (widths
512/384/256/128 for scores, exp and AV), and the remaining triangular edge
is zeroed with a single shared 128x128 0/1 strip on the DVE.  Off-diagonal
upper tiles are skipped entirely.

Scheduling: the AV matmul for score-chunk i is emitted after the score
matmuls of chunk i+1, so its wait on the ACT exp resolves early and the PE
weight loads stay hidden.  Attention for query-block qb interleaves the
projections for chunk qb+1 as PE fillers (the exp-paced attention leaves
the PE underused), and the ACT-heaviest final block absorbs all deferred
output projections; only the last block's Wo remains as a tail.
"""

import os
import sys
import time

sys.path.insert(0, "/opt/trn_rl_repo")

import numpy as np

import concourse.bass as bass
import concourse.mybir as mybir
import concourse.tile as tile
from concourse import bacc
from concourse.bass_utils import run_bass_kernel_spmd
from concourse.masks import make_identity

F16 = mybir.dt.float16
F32 = mybir.dt.float32
P = 128

# Problem constants (full size).
D_MODEL = 1024
NUM_HEADS = 16
DK = D_MODEL // NUM_HEADS  # 64
BATCH = 4
SEQ = 2048
N_CORES = 8

LAST_EXEC_NS = None
LAST_RESULTS = None


def build_program(seq=SEQ, d_model=D_MODEL, num_heads=NUM_HEADS, mode="causal"):
    """Build the per-core Bass program.  Uniform across cores (SPMD).

    mode: "causal" (tril mask, block-skip + partial-width diagonal),
          "dense"  (no mask),
          "mask"   (arbitrary 0/1 mask, multiplicative, streamed from DRAM).
    """
    assert d_model % 256 == 0 and seq % P == 0
    HL = num_heads // 2              # local heads per core
    PAIRS = HL // 2                  # head-pairs (128 partitions each)
    FL = HL * DK                     # local features (columns of W slices)
    IN_T = d_model // P              # input-dim tiles
    FT = FL // P                     # local f tiles
    TT = seq // P                    # token tiles
    QBS = min(512, seq)              # qi block size
    QB = seq // QBS                  # qi blocks
    KJ = seq // P                    # key tiles
    KPB = QBS // P                   # key tiles per qi block (diag width)
    OFC = (d_model + 511) // 512     # output-feature chunks
    OFS = min(512, d_model)
    assert PAIRS >= 1 and FT >= 1 and QB >= 1

    nc = bacc.Bacc()
    # x and weights are pre-swizzled on the host to the SBUF layout so every
    # DMA moves 4-8 KiB contiguous per partition (strided 1KiB-row gathers
    # cap the DMA engines well below peak).
    xtq = nc.declare_dram_parameter("xtq", [P, QB, IN_T, QBS], F16,
                                    isOutput=False)
    xtk = nc.declare_dram_parameter("xtk", [P, QB, IN_T, QBS], F16,
                                    isOutput=False)
    xtv = nc.declare_dram_parameter("xtv", [P, QB, IN_T, QBS], F16,
                                    isOutput=False)
    wqt = nc.declare_dram_parameter("wqt", [P, IN_T, FL], F16, isOutput=False)
    wkt = nc.declare_dram_parameter("wkt", [P, IN_T, FL], F16, isOutput=False)
    wvt = nc.declare_dram_parameter("wvt", [P, IN_T, FL], F16, isOutput=False)
    bqr = nc.declare_dram_parameter("bqr", [P, PAIRS], F32, isOutput=False)
    bkr = nc.declare_dram_parameter("bkr", [P, PAIRS], F32, isOutput=False)
    bvrow = nc.declare_dram_parameter("bvrow", [1, FL], F16, isOutput=False)
    wot = nc.declare_dram_parameter("wot", [P, FT, d_model], F16,
                                    isOutput=False)
    if mode == "mask":
        maskt = nc.declare_dram_parameter("maskt", [seq, seq], F16, isOutput=False)
    out = nc.declare_dram_parameter("out", [seq, d_model], F16, isOutput=True)

    AF = mybir.ActivationFunctionType

    with tile.TileContext(nc) as tc:
        with (
            tc.tile_pool(name="const", bufs=1) as cpool,
            tc.tile_pool(name="big", bufs=1) as big,
            tc.tile_pool(name="xs", bufs=2) as xs,
            tc.tile_pool(name="es", bufs=6) as esp,
            tc.tile_pool(name="ep", bufs=2) as epi,
            tc.tile_pool(name="osb", bufs=3) as osb,
        ):
            # ---- constants.  Biases first (tiny, gate DVE adds), then the
            # weight/x halves interleaved in exactly PE consumption order so
            # the first K-projection can start as early as possible.  wo is
            # deferred past the chunk-1 prefetch (only needed by Wo fillers).
            bqr_sb = cpool.tile([P, PAIRS], F32)
            nc.sync.dma_start(bqr_sb[:], bqr[:, :])
            bkr_sb = cpool.tile([P, PAIRS], F32)
            nc.sync.dma_start(bkr_sb[:], bkr[:, :])
            bv_sb = cpool.tile([1, FL], F16)
            nc.sync.dma_start(bv_sb[:], bvrow[:, :])
            wk_sb = cpool.tile([P, IN_T, FL], F16)
            wq_sb = cpool.tile([P, IN_T, FL], F16)
            wv_sb = cpool.tile([P, IN_T, FL], F16)
            H_IT = IN_T // 2
            xk0 = xs.tile([P, IN_T, QBS], F16, tag="xk", name="xk_0")
            xq0 = xs.tile([P, IN_T, QBS], F16, tag="xq", name="xq_0")
            xv0 = xs.tile([P, IN_T, QBS], F16, tag="xv", name="xv_0")
            # split the setup burst across both HWDGE queues (SP carries K
            # then V, ACT carries Q) — one queue caps well below HBM
            # bandwidth and the start is DMA-bound; ACT is idle until the
            # first exp.
            for eng, w_sb, _wr, x_t, _xr in (
                    (nc.sync, wk_sb, wkt, xk0, xtk),
                    (nc.scalar, wq_sb, wqt, xq0, xtq),
                    (nc.scalar, wv_sb, wvt, xv0, xtv)):
                eng.dma_start(w_sb[:, 0:H_IT, :], _wr[:, 0:H_IT, :])
                eng.dma_start(x_t[:, 0:H_IT, :], _xr[:, 0, 0:H_IT, :])
                eng.dma_start(w_sb[:, H_IT:IN_T, :], _wr[:, H_IT:IN_T, :])
                eng.dma_start(x_t[:, H_IT:IN_T, :], _xr[:, 0, H_IT:IN_T, :])
            wo_sb = cpool.tile([P, FT, d_model], F16)

            def emit_wo_dma():
                nc.sync.dma_start(wo_sb[:], wot[:, :, :])

            ones1 = cpool.tile([1, P], F16)
            nc.gpsimd.memset(ones1[:], 1.0)
            bvb = cpool.tile([P, FL], F16)
            nc.gpsimd.partition_broadcast(bvb[:], bv_sb[0:1, :])
            # shared 128x128 lower-triangular 0/1 strip: keep t >= k
            tri = cpool.tile([P, P], F16)
            nc.gpsimd.memset(tri[:], 1.0)
            nc.gpsimd.affine_select(
                out=tri[:], in_=tri[:],
                compare_op=mybir.AluOpType.is_ge,
                fill=0.0, base=0,
                pattern=[[1, P]], channel_multiplier=-1)

            # ---- persistent activations ----
            VW = DK + 2  # V columns + softmax-ones + pad (small LDWEIGHTS)
            qT_sb = big.tile([P, PAIRS, seq], F16)   # [2-head f, pair, tok]
            kT_sb = big.tile([P, PAIRS, seq], F16)
            v_sb = big.tile([P, TT, HL, VW], F16)  # [tok_in_tile, kj, h, d|1]
            oT_sb = big.tile([P, FT, seq], F16)      # attention out, f-major

            nc.gpsimd.memset(v_sb[:], 0.0)
            nc.gpsimd.memset(v_sb[:, :, :, DK:DK + 1], 1.0)

            # warm the ACT exp table early (one-time ~2.7us load)
            es_warm = esp.tile([1, 8], F16, tag="warm")
            nc.scalar.activation(es_warm[:], ones1[0:1, 0:8], AF.Exp, scale=1.0)

            # PSUM: scores get their own 2-deep ring of [128,2,512] tiles
            # (tag "s", 4 banks) and the projection/Wo filler pieces a
            # separate 2-deep ring of single-bank [128,512] tiles (tag "f",
            # 2 banks), so a filler's PSUM reuse never blocks the score
            # pipeline (and vice versa); + attention out tag "o" (2 banks).
            pool_cm = tc.tile_pool(name="pmm", bufs=2, space="PSUM")
            pmm = pool_cm.__enter__()
            fpool_cm = tc.tile_pool(name="pf", bufs=2, space="PSUM")
            pfil = fpool_cm.__enter__()
            opool_cm = tc.tile_pool(name="po", bufs=2, space="PSUM")
            pop = opool_cm.__enter__()

            def emit_proj_dma(ch):
                    xk_t = xs.tile([P, IN_T, QBS], F16, tag="xk",
                                   name=f"xk_{ch}")
                    nc.sync.dma_start(xk_t[:], xtk[:, ch, :, :])
                    xq_t = xs.tile([P, IN_T, QBS], F16, tag="xq",
                                   name=f"xq_{ch}")
                    nc.sync.dma_start(xq_t[:], xtq[:, ch, :, :])
                    xv_t = xs.tile([P, IN_T, QBS], F16, tag="xv",
                                   name=f"xv_{ch}")
                    nc.sync.dma_start(xv_t[:], xtv[:, ch, :, :])
                    return xk_t, xq_t, xv_t

            def proj_pieces(ch, tiles=None):
                    tsl = slice(ch * QBS, (ch + 1) * QBS)
                    xk_t, xq_t, xv_t = tiles if tiles else emit_proj_dma(ch)
                    pieces = []
                    def kq_piece(pair, which):
                        def go():
                            _emit_kq_piece(ch, tsl, xk_t if which == 0 else xq_t,
                                           pair, which)
                        return go
                    def v_piece(tl):
                        def go():
                            _emit_v_one(ch, xv_t, tl)
                        return go
                    for pair in range(PAIRS):
                        pieces.append(kq_piece(pair, 0))
                        pieces.append(kq_piece(pair, 1))
                    for tl in range(KPB):
                        pieces.append(v_piece(tl))
                    return pieces

            def emit_proj(ch, tiles=None):
                    for piece in proj_pieces(ch, tiles):
                        piece()

            def _emit_kq_piece(ch, tsl, x_t, pair, which):
                        fsl = slice(pair * P, (pair + 1) * P)
                        w_sb = wk_sb if which == 0 else wq_sb
                        dst = kT_sb if which == 0 else qT_sb
                        br = bkr_sb if which == 0 else bqr_sb
                        ps = pfil.tile([P, QBS], F32, tag="f",
                                       name=f"kq_{ch}_{pair}_{which}")
                        for it in range(IN_T):
                            nc.tensor.matmul(ps[:], w_sb[:, it, fsl],
                                             x_t[:, it, :],
                                             start=(it == 0), stop=(it == IN_T - 1))
                        nc.vector.tensor_scalar_add(dst[:, pair, tsl], ps[:],
                                                    br[:, pair:pair + 1])

            def _emit_v_one(ch, xv_t, tl):
                        tt = ch * KPB + tl
                        v_ps = pfil.tile([P, QBS], F32, tag="f",
                                         name=f"v_{ch}_{tl}")
                        for it in range(IN_T):
                            nc.tensor.matmul(
                                v_ps[:, 0:FL],
                                xv_t[:, it, tl * P:(tl + 1) * P],
                                wv_sb[:, it, :],
                                start=(it == 0), stop=(it == IN_T - 1))
                        nc.vector.tensor_tensor(
                            v_sb[:, tt, :, 0:DK],
                            v_ps[:, 0:FL].rearrange("p (h d) -> p h d", h=HL),
                            bvb[:].rearrange("p (h d) -> p h d", h=HL),
                            mybir.AluOpType.add)

            # ---- attention for qi block qb, all local heads ----
            # Chunk specs: list of [(kj, q_off), ...] (1-2 entries).  On the
            # causal diagonal q_off restricts scores/exp/AV to columns at or
            # past the causal boundary of that key tile.
            def attn_chunk_specs(qb):
                specs = []
                if mode == "causal":
                    d = qb * KPB
                    for j0 in range(0, KPB, 2):
                        specs.append([(d + j0, j0 * P), (d + j0 + 1, (j0 + 1) * P)])
                    for kj0 in range(0, qb * KPB, 2):
                        specs.append([(kj0, 0), (kj0 + 1, 0)])
                else:
                    for kj0 in range(0, KJ, 2):
                        if kj0 + 1 < KJ:
                            specs.append([(kj0, 0), (kj0 + 1, 0)])
                        else:
                            specs.append([(kj0, 0)])
                return specs

            def emit_attn(qb, fillers=()):
                fillers = list(fillers)
                qsl = slice(qb * QBS, (qb + 1) * QBS)
                specs = attn_chunk_specs(qb)
                n_chunks = len(specs)
                # The softmax-sum normalisation of head h is emitted one head
                # late (during head h+1's first chunks): its DVE chain would
                # otherwise head-of-line-block the next head's causal-edge
                # multiplies in the strict-FIFO DVE queue, stalling the PE.
                pending_norm = [None]

                def emit_norm():
                    if pending_norm[0] is None:
                        return
                    hh, o_prev = pending_norm[0]
                    pending_norm[0] = None
                    pair_p = hh // 2
                    po_p = (hh % 2) * DK
                    # (the sum is copied to SBUF first: the custom-DVE fast
                    # reciprocal misbehaves on PSUM operands on hardware)
                    srow = epi.tile([1, QBS], F32, tag="srow")
                    nc.vector.tensor_copy(srow[:], o_prev[DK:DK + 1, :])
                    recip_row = epi.tile([1, QBS], F32, tag="recip_row")
                    nc.vector.reciprocal_approx_fast(recip_row[:], srow[:])
                    recipb = epi.tile([DK, QBS], F32, tag="recipb")
                    nc.gpsimd.partition_broadcast(recipb[:], recip_row[0:1, :])
                    nc.vector.tensor_mul(oT_sb[po_p:po_p + DK, pair_p, qsl],
                                         o_prev[0:DK, :], recipb[:])

                for h in range(HL):
                    pair = h // 2
                    po = (h % 2) * DK
                    o_ps = pop.tile([P, QBS], F32, tag="o",
                                    name=f"o_{qb}_{h}")

                    def emit_scores(ci):
                        spec = specs[ci]
                        s_ps = pmm.tile([P, 2, QBS], F32, tag="s",
                                        name=f"s_{qb}_{h}_{ci}")
                        es = esp.tile([P, 2, QBS], F16, tag="es",
                                      name=f"es_{qb}_{h}_{ci}")
                        # scores are computed over the chunk's widest column
                        # range so a single fused exp can cover the whole
                        # chunk (fewer ACT instructions — exp paces the busy
                        # attention phases); AV still skips the columns the
                        # causal mask fully excludes per key tile.
                        off0 = min(off for _, off in spec)
                        for i, (kj, off) in enumerate(spec):
                            nc.tensor.matmul(
                                s_ps[:, i, off0:QBS],
                                kT_sb[po:po + DK, pair, kj * P:(kj + 1) * P],
                                qT_sb[po:po + DK, pair,
                                      qb * QBS + off0:(qb + 1) * QBS],
                                start=True, stop=True)
                        n = len(spec)
                        nc.scalar.activation(es[:, :n, off0:QBS],
                                             s_ps[:, :n, off0:QBS],
                                             AF.Exp, scale=0.125)
                        for i, (kj, off) in enumerate(spec):
                            if mode == "causal" and kj // KPB == qb:
                                j = kj % KPB
                                csl = slice(j * P, (j + 1) * P)
                                nc.vector.tensor_mul(es[:, i, csl],
                                                     es[:, i, csl], tri[:])
                            elif mode == "mask":
                                m_t = esp.tile([P, QBS], F16, tag="mt")
                                nc.sync.dma_start(
                                    m_t[:], maskt[kj * P:(kj + 1) * P, qsl])
                                nc.vector.tensor_mul(es[:, i, :], es[:, i, :],
                                                     m_t[:])
                        return es

                    def emit_av(ci, es):
                        spec = specs[ci]
                        for i, (kj, off) in enumerate(spec):
                            nc.tensor.matmul(
                                o_ps[0:VW, off:QBS], v_sb[:, kj, h, :],
                                es[:, i, off:QBS],
                                start=(ci == 0 and i == 0),
                                stop=(ci == n_chunks - 1 and i == len(spec) - 1))

                    # software pipeline: AV for chunk i after scores of i+1,
                    # so the exp wait resolves while the PE streams scores.
                    # Fillers go between the scores and the first AV: their PE
                    # work covers the latency of the DVE queue (causal-edge
                    # multiplies, delayed norm) that the first AV waits on.
                    quota = -(-len(fillers) // (HL - h))  # ceil: spread evenly
                    prev_es = emit_scores(0)
                    if n_chunks == 1:
                        emit_norm()
                        while quota > 0 and fillers:
                            fillers.pop(0)()
                            quota -= 1
                    for ci in range(1, n_chunks):
                        es = emit_scores(ci)
                        if ci == 1:
                            emit_norm()
                        if quota > 0 and fillers:
                            fillers.pop(0)()
                            quota -= 1
                        emit_av(ci - 1, prev_es)
                        prev_es = es
                    emit_av(n_chunks - 1, prev_es)
                    while quota > 0 and fillers:
                        fillers.pop(0)()
                        quota -= 1
                    pending_norm[0] = (h, o_ps)
                emit_norm()
                for f in fillers:
                    f()

            # ---- output projection for one token chunk ----
            def wo_pieces(qb, use_act=False):
                def tt_piece(tl, ofc):
                    def go():
                        _emit_wo_tt(qb, tl, ofc, use_act and ofc == 1)
                    return go
                return [tt_piece(tl, ofc)
                        for tl in range(KPB) for ofc in range(OFC)]

            def emit_wo(qb, use_act=False):
                for piece in wo_pieces(qb, use_act):
                    piece()

            def _emit_wo_tt(qb, tl, ofc, act_copy=False):
                    tt = qb * KPB + tl
                    osl = slice(ofc * OFS, (ofc + 1) * OFS)
                    w_ps = pfil.tile([P, QBS], F32, tag="f",
                                     name=f"w_{tt}_{ofc}")
                    for ft in range(FT):
                        nc.tensor.matmul(w_ps[:, 0:OFS],
                                         oT_sb[:, ft, tt * P:(tt + 1) * P],
                                         wo_sb[:, ft, osl],
                                         start=(ft == 0), stop=(ft == FT - 1))
                    o_out = osb.tile([P, OFS], F16, tag="oo")
                    if act_copy:
                        # tail only: ACT is idle there, halves the PSUM
                        # evacuation latency (Copy shares the Exp table)
                        nc.scalar.copy(o_out[:], w_ps[:, 0:OFS])
                    else:
                        nc.vector.tensor_copy(o_out[:], w_ps[:, 0:OFS])
                    nc.sync.dma_start(out[tt * P:(tt + 1) * P, osl], o_out[:])

            if mode == "causal":
                # x-chunk DMA runs one full attention phase ahead of the
                # projection fillers that consume it (chunk 1 right behind
                # the setup DMAs), so the PE stream never blocks on it.
                prefetched = {}
                if QB > 1:
                    prefetched[1] = emit_proj_dma(1)
                emit_wo_dma()
                # head 0 of attention block 0 only needs pair 0's K/Q and the
                # V tiles of chunk 0 — start it while the rest of the setup
                # DMA is still in flight and fold the remaining projection
                # pieces into attn0's fillers.
                p0 = proj_pieces(0, (xk0, xq0, xv0))
                nkq = 2 * PAIRS
                cut = min(4, nkq)
                head0_pieces = p0[0:cut] + p0[nkq:]  # K0,Q0,K1,Q1 + all V
                later_pieces = p0[cut:nkq]           # K2,Q2,K3,Q3...
                for piece in head0_pieces:
                    piece()
                for qb in range(QB):
                    if qb + 2 < QB:
                        prefetched[qb + 2] = emit_proj_dma(qb + 2)
                    fillers = []
                    if qb == 0:
                        fillers += later_pieces
                    if qb + 1 < QB:
                        fillers += proj_pieces(qb + 1, prefetched.pop(qb + 1))
                    if qb == QB - 1:
                        for w in range(QB - 1):
                            fillers += wo_pieces(w)
                    emit_attn(qb, fillers)
                emit_wo(QB - 1, use_act=True)
            else:
                emit_wo_dma()
                for ch in range(QB):
                    emit_proj(ch, (xk0, xq0, xv0) if ch == 0 else None)
                for qb in range(QB):
                    emit_attn(qb)
                    emit_wo(qb)

            opool_cm.__exit__(None, None, None)
            fpool_cm.__exit__(None, None, None)
            pool_cm.__exit__(None, None, None)

    nc.compile()
    return nc


_PROGRAMS = {}


def _get_program(mode, seq=SEQ, d_model=D_MODEL, num_heads=NUM_HEADS):
    key = (mode, seq, d_model, num_heads)
    if key not in _PROGRAMS:
        _PROGRAMS[key] = build_program(seq, d_model, num_heads, mode)
    return _PROGRAMS[key]


def _detect_mode(mask, seq):
    m = np.asarray(mask)
    if (m != 0).all():
        return "dense"
    tril = np.tril(np.ones((seq, seq), np.int8))
    if np.array_equal((m != 0).astype(np.int8), tril):
        return "causal"
    return "mask"


def _swizzle_x(X, seq, d_model):
    # [seq, d_model] -> [P, QB, IN_T, QBS]: x[p, ch, it, t] = X[ch*QBS+t,
    # it*P+p], so each per-chunk DMA is contiguous per partition.
    QBS = min(512, seq)
    QB = seq // QBS
    IN_T = d_model // P
    return np.ascontiguousarray(
        X.reshape(QB, QBS, IN_T, P).transpose(3, 0, 2, 1)).astype(np.float16)


def _swizzle_w(Whalf, d_model, FL):
    # [FL, d_model] -> [P, IN_T, FL]: w[p, it, f] = Whalf[f, it*P+p]
    IN_T = d_model // P
    return np.ascontiguousarray(
        Whalf.reshape(FL, IN_T, P).transpose(2, 1, 0)).astype(np.float16)


def prep_inputs(Q, K, V, mask, Wq, bq, Wk, bk, Wv, bv, Wo, bo,
                num_heads=NUM_HEADS, mode=None):
    batch, seq, d_model = Q.shape
    HL = num_heads // 2
    FL = HL * (d_model // num_heads)
    FT = FL // P
    PAIRS = HL // 2
    if mode is None:
        mode = _detect_mode(mask, seq)
    maskt = None
    if mode == "mask":
        maskt = np.ascontiguousarray(
            (np.asarray(mask) != 0).astype(np.float16).T)
    in_maps = []
    for b in range(batch):
        xtq = _swizzle_x(np.asarray(Q[b], np.float32), seq, d_model)
        xtk = _swizzle_x(np.asarray(K[b], np.float32), seq, d_model)
        xtv = _swizzle_x(np.asarray(V[b], np.float32), seq, d_model)
        for half in range(2):
            fsl = slice(half * FL, (half + 1) * FL)
            # wot[p, ft, o] = Wo[o, fsl][ft*P+p] (lhsT rows of Wo^T)
            wot = np.ascontiguousarray(
                Wo[:, fsl].T.reshape(FT, P, d_model).transpose(1, 0, 2)
            ).astype(np.float16)
            im = {
                "xtq": xtq, "xtk": xtk, "xtv": xtv,
                "wqt": _swizzle_w(Wq[fsl, :], d_model, FL),
                "wkt": _swizzle_w(Wk[fsl, :], d_model, FL),
                "wvt": _swizzle_w(Wv[fsl, :], d_model, FL),
                "bqr": np.ascontiguousarray(
                    bq[fsl].reshape(PAIRS, P).T).astype(np.float32),
                "bkr": np.ascontiguousarray(
                    bk[fsl].reshape(PAIRS, P).T).astype(np.float32),
                "bvrow": bv[fsl].reshape(1, FL).astype(np.float16),
                "wot": wot,
            }
            if maskt is not None:
                im["maskt"] = maskt
            in_maps.append(im)
    return in_maps, mode


def _install_trace_hooks():
    """Provide antenv.axon_hooks (missing in this image) so that
    run_bass_kernel_spmd(trace=True) can capture NTFF profiles via the
    axon PJRT .so.  Bench-only; the graded path never enables tracing."""
    import contextlib
    import ctypes
    import types
    try:
        from antenv import axon_hooks  # noqa: F401
        return
    except ImportError:
        pass
    lib = ctypes.CDLL("/opt/axon/libaxon_pjrt.so")
    if not hasattr(lib, "axon_start_nrt_profile"):
        return
    lib.axon_start_nrt_profile.argtypes = [ctypes.POINTER(ctypes.c_int64),
                                           ctypes.c_size_t]
    lib.axon_start_nrt_profile.restype = ctypes.c_int64
    lib.axon_stop_nrt_profile.argtypes = [ctypes.c_char_p]
    lib.axon_stop_nrt_profile.restype = ctypes.c_int64

    @contextlib.contextmanager
    def _hook(output_dir, device_ids):
        import jax
        jax.devices()
        if device_ids:
            ids = (ctypes.c_int64 * len(device_ids))(*device_ids)
            rc = lib.axon_start_nrt_profile(ids, len(device_ids))
        else:
            rc = lib.axon_start_nrt_profile(None, 0)
        if rc != 0:
            raise RuntimeError(f"axon_start_nrt_profile rc={rc}")
        try:
            yield
        finally:
            n = lib.axon_stop_nrt_profile(str(output_dir).encode())
            print(f"profile: {n} file(s) written to {output_dir}", file=sys.stderr)

    mod = types.ModuleType("antenv.axon_hooks")
    mod.get_axon_ntff_profile_hook = lambda: _hook
    mod.set_axon_ntff_profile_hook = lambda h: None
    sys.modules["antenv.axon_hooks"] = mod
    import concourse.bass_utils as bu
    bu.upload_artifacts = lambda tmpdir: f"local:{tmpdir}"


def kernel(Q, K, V, mask, Wq, bq, Wk, bk, Wv, bv, Wo, bo):
    global LAST_EXEC_NS, LAST_RESULTS
    Q = np.asarray(Q); K = np.asarray(K); V = np.asarray(V)
    mask = np.asarray(mask)
    Wq = np.asarray(Wq, np.float32); bq = np.asarray(bq, np.float32)
    Wk = np.asarray(Wk, np.float32); bk = np.asarray(bk, np.float32)
    Wv = np.asarray(Wv, np.float32); bv = np.asarray(bv, np.float32)
    Wo = np.asarray(Wo, np.float32); bo = np.asarray(bo, np.float32)
    batch, seq, d_model = Q.shape

    in_maps, mode = prep_inputs(Q, K, V, mask, Wq, bq, Wk, bk, Wv, bv, Wo, bo)
    nc = _get_program(mode, seq, d_model, NUM_HEADS)

    trace = bool(os.environ.get("KBENCH_TRACE"))
    tmpdir = os.environ.get("KBENCH_TRACE_DIR") or None
    if trace:
        _install_trace_hooks()
    res = run_bass_kernel_spmd(nc, in_maps, list(range(N_CORES)), trace=trace,
                               tmpdir=tmpdir)
    LAST_EXEC_NS = res.exec_time_ns
    LAST_RESULTS = res
    out = np.empty((batch, seq, d_model), np.float32)
    for b in range(batch):
        out[b] = (res.results[2 * b]["out"].astype(np.float32)
                  + res.results[2 * b + 1]["out"].astype(np.float32) + bo)
    return out


# revision 37
# speedup vs baseline: 1.0025x; 1.0025x over previous
"""Multi-head attention TRN2 kernel (8 NeuronCores).

Sharding: core (2b + h2) handles batch b (of 4) and head-half h2 (8 of 16
heads).  Each core projects its batch's Q/K/V through its 512-column slice
of Wq/Wk/Wv, runs causal flash-attention for its 8 heads, and computes a
partial output projection through its 512 rows of Wo^T.  The two partial
outputs per batch are summed on the host (the "all-reduce after W_o"),
plus the output bias.

All matmuls run in fp16 with fp32 PSUM accumulation.  Scores are computed
transposed (S^T[kj, qi] = kT.T @ qT) so the softmax sum comes for free from
a ones-column appended to V (padded to 128 columns so LDWEIGHTS gets fast
weight load), and the attention output lands f-major, which is exactly the
lhsT layout the Wo matmul needs.  Normalisation (divide by the softmax sum)
is a DVE fast-reciprocal + GPSIMD partition-broadcast + DVE multiply.

Causal handling: on the 4 diagonal key-tiles of each 512-token query block
only the q-columns at or past the causal boundary are computed (widths
512/384/256/128 for scores, exp and AV), and the remaining triangular edge
is zeroed with a single shared 128x128 0/1 strip on the DVE.  Off-diagonal
upper tiles are skipped entirely.

Scheduling: the AV matmul for score-chunk i is emitted after the score
matmuls of chunk i+1, so its wait on the ACT exp resolves early and the PE
weight loads stay hidden.  Attention for query-block qb interleaves the
projections for chunk qb+1 as PE fillers (the exp-paced attention leaves
the PE underused), and the ACT-heaviest final block absorbs all deferred
output projections; only the last block's Wo remains as a tail.
"""

import os
import sys
import time

sys.path.insert(0, "/opt/trn_rl_repo")

import numpy as np

import concourse.bass as bass
import concourse.mybir as mybir
import concourse.tile as tile
from concourse import bacc
from concourse.bass_utils import run_bass_kernel_spmd
from concourse.masks import make_identity

F16 = mybir.dt.float16
F32 = mybir.dt.float32
P = 128

# Problem constants (full size).
D_MODEL = 1024
NUM_HEADS = 16
DK = D_MODEL // NUM_HEADS  # 64
BATCH = 4
SEQ = 2048
N_CORES = 8

LAST_EXEC_NS = None
LAST_RESULTS = None


def build_program(seq=SEQ, d_model=D_MODEL, num_heads=NUM_HEADS, mode="causal"):
    """Build the per-core Bass program.  Uniform across cores (SPMD).

    mode: "causal" (tril mask, block-skip + partial-width diagonal),
          "dense"  (no mask),
          "mask"   (arbitrary 0/1 mask, multiplicative, streamed from DRAM).
    """
    assert d_model % 256 == 0 and seq % P == 0
    HL = num_heads // 2              # local heads per core
    PAIRS = HL // 2                  # head-pairs (128 partitions each)
    FL = HL * DK                     # local features (columns of W slices)
    IN_T = d_model // P              # input-dim tiles
    FT = FL // P                     # local f tiles
    TT = seq // P                    # token tiles
    QBS = min(512, seq)              # qi block size
    QB = seq // QBS                  # qi blocks
    KJ = seq // P                    # key tiles
    KPB = QBS // P                   # key tiles per qi block (diag width)
    OFC = (d_model + 511) // 512     # output-feature chunks
    OFS = min(512, d_model)
    assert PAIRS >= 1 and FT >= 1 and QB >= 1

    nc = bacc.Bacc()
    # x and weights are pre-swizzled on the host to the SBUF layout so every
    # DMA moves 4-8 KiB contiguous per partition (strided 1KiB-row gathers
    # cap the DMA engines well below peak).
    xtq = nc.declare_dram_parameter("xtq", [P, QB, IN_T, QBS], F16,
                                    isOutput=False)
    xtk = nc.declare_dram_parameter("xtk", [P, QB, IN_T, QBS], F16,
                                    isOutput=False)
    xtv = nc.declare_dram_parameter("xtv", [P, QB, IN_T, QBS], F16,
                                    isOutput=False)
    wqt = nc.declare_dram_parameter("wqt", [P, IN_T, FL], F16, isOutput=False)
    wkt = nc.declare_dram_parameter("wkt", [P, IN_T, FL], F16, isOutput=False)
    wvt = nc.declare_dram_parameter("wvt", [P, IN_T, FL], F16, isOutput=False)
    bqr = nc.declare_dram_parameter("bqr", [P, PAIRS], F32, isOutput=False)
    bkr = nc.declare_dram_parameter("bkr", [P, PAIRS], F32, isOutput=False)
    bvrow = nc.declare_dram_parameter("bvrow", [1, FL], F16, isOutput=False)
    wot = nc.declare_dram_parameter("wot", [P, FT, d_model], F16,
                                    isOutput=False)
    if mode == "mask":
        maskt = nc.declare_dram_parameter("maskt", [seq, seq], F16, isOutput=False)
    out = nc.declare_dram_parameter("out", [seq, d_model], F16, isOutput=True)

    AF = mybir.ActivationFunctionType

    with tile.TileContext(nc) as tc:
        with (
            tc.tile_pool(name="const", bufs=1) as cpool,
            tc.tile_pool(name="big", bufs=1) as big,
            tc.tile_pool(name="xs", bufs=2) as xs,
            tc.tile_pool(name="es", bufs=6) as esp,
            tc.tile_pool(name="ep", bufs=2) as epi,
            tc.tile_pool(name="osb", bufs=3) as osb,
        ):
            # ---- constants.  Biases first (tiny, gate DVE adds), then the
            # weight/x halves interleaved in exactly PE consumption order so
            # the first K-projection can start as early as possible.  wo is
            # deferred past the chunk-1 prefetch (only needed by Wo fillers).
            bqr_sb = cpool.tile([P, PAIRS], F32)
            nc.sync.dma_start(bqr_sb[:], bqr[:, :])
            bkr_sb = cpool.tile([P, PAIRS], F32)
            nc.sync.dma_start(bkr_sb[:], bkr[:, :])
            bv_sb = cpool.tile([1, FL], F16)
            nc.sync.dma_start(bv_sb[:], bvrow[:, :])
            wk_sb = cpool.tile([P, IN_T, FL], F16)
            wq_sb = cpool.tile([P, IN_T, FL], F16)
            wv_sb = cpool.tile([P, IN_T, FL], F16)
            H_IT = IN_T // 2
            xk0 = xs.tile([P, IN_T, QBS], F16, tag="xk", name="xk_0")
            xq0 = xs.tile([P, IN_T, QBS], F16, tag="xq", name="xq_0")
            xv0 = xs.tile([P, IN_T, QBS], F16, tag="xv", name="xv_0")
            # split the setup burst across both HWDGE queues (SP carries K
            # then V, ACT carries Q) — one queue caps well below HBM
            # bandwidth and the start is DMA-bound; ACT is idle until the
            # first exp.
            for eng, w_sb, _wr, x_t, _xr in (
                    (nc.sync, wk_sb, wkt, xk0, xtk),
                    (nc.scalar, wq_sb, wqt, xq0, xtq)):
                eng.dma_start(w_sb[:, 0:H_IT, :], _wr[:, 0:H_IT, :])
                eng.dma_start(x_t[:, 0:H_IT, :], _xr[:, 0, 0:H_IT, :])
                eng.dma_start(w_sb[:, H_IT:IN_T, :], _wr[:, H_IT:IN_T, :])
                eng.dma_start(x_t[:, H_IT:IN_T, :], _xr[:, 0, H_IT:IN_T, :])
            # V halves split across both queues so V lands right when the
            # first AV needs it
            nc.sync.dma_start(wv_sb[:, 0:H_IT, :], wvt[:, 0:H_IT, :])
            nc.sync.dma_start(xv0[:, 0:H_IT, :], xtv[:, 0, 0:H_IT, :])
            nc.scalar.dma_start(wv_sb[:, H_IT:IN_T, :], wvt[:, H_IT:IN_T, :])
            nc.scalar.dma_start(xv0[:, H_IT:IN_T, :], xtv[:, 0, H_IT:IN_T, :])
            wo_sb = cpool.tile([P, FT, d_model], F16)

            def emit_wo_dma():
                nc.sync.dma_start(wo_sb[:], wot[:, :, :])

            ones1 = cpool.tile([1, P], F16)
            nc.gpsimd.memset(ones1[:], 1.0)
            bvb = cpool.tile([P, FL], F16)
            nc.gpsimd.partition_broadcast(bvb[:], bv_sb[0:1, :])
            # shared 128x128 lower-triangular 0/1 strip: keep t >= k
            tri = cpool.tile([P, P], F16)
            nc.gpsimd.memset(tri[:], 1.0)
            nc.gpsimd.affine_select(
                out=tri[:], in_=tri[:],
                compare_op=mybir.AluOpType.is_ge,
                fill=0.0, base=0,
                pattern=[[1, P]], channel_multiplier=-1)

            # ---- persistent activations ----
            VW = DK + 2  # V columns + softmax-ones + pad (small LDWEIGHTS)
            qT_sb = big.tile([P, PAIRS, seq], F16)   # [2-head f, pair, tok]
            kT_sb = big.tile([P, PAIRS, seq], F16)
            v_sb = big.tile([P, TT, HL, VW], F16)  # [tok_in_tile, kj, h, d|1]
            oT_sb = big.tile([P, FT, seq], F16)      # attention out, f-major

            nc.gpsimd.memset(v_sb[:], 0.0)
            nc.gpsimd.memset(v_sb[:, :, :, DK:DK + 1], 1.0)

            # warm the ACT exp table early (one-time ~2.7us load)
            es_warm = esp.tile([1, 8], F16, tag="warm")
            nc.scalar.activation(es_warm[:], ones1[0:1, 0:8], AF.Exp, scale=1.0)

            # PSUM: scores get their own 2-deep ring of [128,2,512] tiles
            # (tag "s", 4 banks) and the projection/Wo filler pieces a
            # separate 2-deep ring of single-bank [128,512] tiles (tag "f",
            # 2 banks), so a filler's PSUM reuse never blocks the score
            # pipeline (and vice versa); + attention out tag "o" (2 banks).
            pool_cm = tc.tile_pool(name="pmm", bufs=2, space="PSUM")
            pmm = pool_cm.__enter__()
            fpool_cm = tc.tile_pool(name="pf", bufs=2, space="PSUM")
            pfil = fpool_cm.__enter__()
            opool_cm = tc.tile_pool(name="po", bufs=2, space="PSUM")
            pop = opool_cm.__enter__()

            def emit_proj_dma(ch):
                    xk_t = xs.tile([P, IN_T, QBS], F16, tag="xk",
                                   name=f"xk_{ch}")
                    nc.sync.dma_start(xk_t[:], xtk[:, ch, :, :])
                    xq_t = xs.tile([P, IN_T, QBS], F16, tag="xq",
                                   name=f"xq_{ch}")
                    nc.sync.dma_start(xq_t[:], xtq[:, ch, :, :])
                    xv_t = xs.tile([P, IN_T, QBS], F16, tag="xv",
                                   name=f"xv_{ch}")
                    nc.sync.dma_start(xv_t[:], xtv[:, ch, :, :])
                    return xk_t, xq_t, xv_t

            def proj_pieces(ch, tiles=None):
                    tsl = slice(ch * QBS, (ch + 1) * QBS)
                    xk_t, xq_t, xv_t = tiles if tiles else emit_proj_dma(ch)
                    pieces = []
                    def kq_piece(pair, which):
                        def go():
                            _emit_kq_piece(ch, tsl, xk_t if which == 0 else xq_t,
                                           pair, which)
                        return go
                    def v_piece(tl):
                        def go():
                            _emit_v_one(ch, xv_t, tl)
                        return go
                    for pair in range(PAIRS):
                        pieces.append(kq_piece(pair, 0))
                        pieces.append(kq_piece(pair, 1))
                    for tl in range(KPB):
                        pieces.append(v_piece(tl))
                    return pieces

            def emit_proj(ch, tiles=None):
                    for piece in proj_pieces(ch, tiles):
                        piece()

            def _emit_kq_piece(ch, tsl, x_t, pair, which):
                        fsl = slice(pair * P, (pair + 1) * P)
                        w_sb = wk_sb if which == 0 else wq_sb
                        dst = kT_sb if which == 0 else qT_sb
                        br = bkr_sb if which == 0 else bqr_sb
                        ps = pfil.tile([P, QBS], F32, tag="f",
                                       name=f"kq_{ch}_{pair}_{which}")
                        for it in range(IN_T):
                            nc.tensor.matmul(ps[:], w_sb[:, it, fsl],
                                             x_t[:, it, :],
                                             start=(it == 0), stop=(it == IN_T - 1))
                        nc.vector.tensor_scalar_add(dst[:, pair, tsl], ps[:],
                                                    br[:, pair:pair + 1])

            def _emit_v_one(ch, xv_t, tl):
                        tt = ch * KPB + tl
                        v_ps = pfil.tile([P, QBS], F32, tag="f",
                                         name=f"v_{ch}_{tl}")
                        for it in range(IN_T):
                            nc.tensor.matmul(
                                v_ps[:, 0:FL],
                                xv_t[:, it, tl * P:(tl + 1) * P],
                                wv_sb[:, it, :],
                                start=(it == 0), stop=(it == IN_T - 1))
                        nc.vector.tensor_tensor(
                            v_sb[:, tt, :, 0:DK],
                            v_ps[:, 0:FL].rearrange("p (h d) -> p h d", h=HL),
                            bvb[:].rearrange("p (h d) -> p h d", h=HL),
                            mybir.AluOpType.add)

            # ---- attention for qi block qb, all local heads ----
            # Chunk specs: list of [(kj, q_off), ...] (1-2 entries).  On the
            # causal diagonal q_off restricts scores/exp/AV to columns at or
            # past the causal boundary of that key tile.
            def attn_chunk_specs(qb):
                specs = []
                if mode == "causal":
                    d = qb * KPB
                    for j0 in range(0, KPB, 2):
                        specs.append([(d + j0, j0 * P), (d + j0 + 1, (j0 + 1) * P)])
                    for kj0 in range(0, qb * KPB, 2):
                        specs.append([(kj0, 0), (kj0 + 1, 0)])
                else:
                    for kj0 in range(0, KJ, 2):
                        if kj0 + 1 < KJ:
                            specs.append([(kj0, 0), (kj0 + 1, 0)])
                        else:
                            specs.append([(kj0, 0)])
                return specs

            def emit_attn(qb, fillers=()):
                fillers = list(fillers)
                qsl = slice(qb * QBS, (qb + 1) * QBS)
                specs = attn_chunk_specs(qb)
                n_chunks = len(specs)
                # The softmax-sum normalisation of head h is emitted one head
                # late (during head h+1's first chunks): its DVE chain would
                # otherwise head-of-line-block the next head's causal-edge
                # multiplies in the strict-FIFO DVE queue, stalling the PE.
                pending_norm = [None]

                def emit_norm():
                    if pending_norm[0] is None:
                        return
                    hh, o_prev = pending_norm[0]
                    pending_norm[0] = None
                    pair_p = hh // 2
                    po_p = (hh % 2) * DK
                    # (the sum is copied to SBUF first: the custom-DVE fast
                    # reciprocal misbehaves on PSUM operands on hardware)
                    srow = epi.tile([1, QBS], F32, tag="srow")
                    nc.vector.tensor_copy(srow[:], o_prev[DK:DK + 1, :])
                    recip_row = epi.tile([1, QBS], F32, tag="recip_row")
                    nc.vector.reciprocal_approx_fast(recip_row[:], srow[:])
                    recipb = epi.tile([DK, QBS], F32, tag="recipb")
                    nc.gpsimd.partition_broadcast(recipb[:], recip_row[0:1, :])
                    nc.vector.tensor_mul(oT_sb[po_p:po_p + DK, pair_p, qsl],
                                         o_prev[0:DK, :], recipb[:])

                for h in range(HL):
                    pair = h // 2
                    po = (h % 2) * DK
                    o_ps = pop.tile([P, QBS], F32, tag="o",
                                    name=f"o_{qb}_{h}")

                    def emit_scores(ci):
                        spec = specs[ci]
                        s_ps = pmm.tile([P, 2, QBS], F32, tag="s",
                                        name=f"s_{qb}_{h}_{ci}")
                        es = esp.tile([P, 2, QBS], F16, tag="es",
                                      name=f"es_{qb}_{h}_{ci}")
                        # scores are computed over the chunk's widest column
                        # range so a single fused exp can cover the whole
                        # chunk (fewer ACT instructions — exp paces the busy
                        # attention phases); AV still skips the columns the
                        # causal mask fully excludes per key tile.
                        off0 = min(off for _, off in spec)
                        for i, (kj, off) in enumerate(spec):
                            nc.tensor.matmul(
                                s_ps[:, i, off0:QBS],
                                kT_sb[po:po + DK, pair, kj * P:(kj + 1) * P],
                                qT_sb[po:po + DK, pair,
                                      qb * QBS + off0:(qb + 1) * QBS],
                                start=True, stop=True)
                        n = len(spec)
                        nc.scalar.activation(es[:, :n, off0:QBS],
                                             s_ps[:, :n, off0:QBS],
                                             AF.Exp, scale=0.125)
                        for i, (kj, off) in enumerate(spec):
                            if mode == "causal" and kj // KPB == qb:
                                j = kj % KPB
                                csl = slice(j * P, (j + 1) * P)
                                nc.vector.tensor_mul(es[:, i, csl],
                                                     es[:, i, csl], tri[:])
                            elif mode == "mask":
                                m_t = esp.tile([P, QBS], F16, tag="mt")
                                nc.sync.dma_start(
                                    m_t[:], maskt[kj * P:(kj + 1) * P, qsl])
                                nc.vector.tensor_mul(es[:, i, :], es[:, i, :],
                                                     m_t[:])
                        return es

                    def emit_av(ci, es):
                        spec = specs[ci]
                        for i, (kj, off) in enumerate(spec):
                            nc.tensor.matmul(
                                o_ps[0:VW, off:QBS], v_sb[:, kj, h, :],
                                es[:, i, off:QBS],
                                start=(ci == 0 and i == 0),
                                stop=(ci == n_chunks - 1 and i == len(spec) - 1))

                    # software pipeline: AV for chunk i after scores of i+1,
                    # so the exp wait resolves while the PE streams scores.
                    # Fillers go between the scores and the first AV: their PE
                    # work covers the latency of the DVE queue (causal-edge
                    # multiplies, delayed norm) that the first AV waits on.
                    quota = -(-len(fillers) // (HL - h))  # ceil: spread evenly
                    prev_es = emit_scores(0)
                    if n_chunks == 1:
                        emit_norm()
                        while quota > 0 and fillers:
                            fillers.pop(0)()
                            quota -= 1
                    for ci in range(1, n_chunks):
                        es = emit_scores(ci)
                        if ci == 1:
                            emit_norm()
                        if quota > 0 and fillers:
                            fillers.pop(0)()
                            quota -= 1
                        emit_av(ci - 1, prev_es)
                        prev_es = es
                    emit_av(n_chunks - 1, prev_es)
                    while quota > 0 and fillers:
                        fillers.pop(0)()
                        quota -= 1
                    pending_norm[0] = (h, o_ps)
                emit_norm()
                for f in fillers:
                    f()

            # ---- output projection for one token chunk ----
            def wo_pieces(qb, use_act=False):
                def tt_piece(tl, ofc):
                    def go():
                        _emit_wo_tt(qb, tl, ofc, use_act and ofc == 1)
                    return go
                return [tt_piece(tl, ofc)
                        for tl in range(KPB) for ofc in range(OFC)]

            def emit_wo(qb, use_act=False):
                for piece in wo_pieces(qb, use_act):
                    piece()

            def _emit_wo_tt(qb, tl, ofc, act_copy=False):
                    tt = qb * KPB + tl
                    osl = slice(ofc * OFS, (ofc + 1) * OFS)
                    w_ps = pfil.tile([P, QBS], F32, tag="f",
                                     name=f"w_{tt}_{ofc}")
                    for ft in range(FT):
                        nc.tensor.matmul(w_ps[:, 0:OFS],
                                         oT_sb[:, ft, tt * P:(tt + 1) * P],
                                         wo_sb[:, ft, osl],
                                         start=(ft == 0), stop=(ft == FT - 1))
                    o_out = osb.tile([P, OFS], F16, tag="oo")
                    if act_copy:
                        # tail only: ACT is idle there, halves the PSUM
                        # evacuation latency (Copy shares the Exp table)
                        nc.scalar.copy(o_out[:], w_ps[:, 0:OFS])
                    else:
                        nc.vector.tensor_copy(o_out[:], w_ps[:, 0:OFS])
                    nc.sync.dma_start(out[tt * P:(tt + 1) * P, osl], o_out[:])

            if mode == "causal":
                # x-chunk DMA runs one full attention phase ahead of the
                # projection fillers that consume it (chunk 1 right behind
                # the setup DMAs), so the PE stream never blocks on it.
                prefetched = {}
                if QB > 1:
                    prefetched[1] = emit_proj_dma(1)
                emit_wo_dma()
                # pair-0's chunk-0 K/Q projections run as interleaved
                # half-depth accumulations in DMA-arrival order (K h1, Q h1,
                # K h2, Q h2) so K's missing second weight half never
                # head-of-line blocks Q whose data already arrived; later
                # pairs reuse the same tiles and run whole.
                tsl0 = slice(0, QBS)
                kq0 = {}
                for half in range(2):
                    isl = range(0, H_IT) if half == 0 else range(H_IT, IN_T)
                    for which, x_t, w_sb in ((0, xk0, wk_sb),
                                             (1, xq0, wq_sb)):
                        if half == 0:
                            ps = pfil.tile([P, QBS], F32, tag="f",
                                           name=f"kq0_0_{which}")
                            kq0[which] = ps
                        else:
                            ps = kq0[which]
                        for it in isl:
                            nc.tensor.matmul(
                                ps[:], w_sb[:, it, 0:P], x_t[:, it, :],
                                start=(it == 0), stop=(it == IN_T - 1))
                        if half == 1:
                            dst = kT_sb if which == 0 else qT_sb
                            br = bkr_sb if which == 0 else bqr_sb
                            nc.vector.tensor_scalar_add(
                                dst[:, 0, tsl0], ps[:], br[:, 0:1])
                for pair in range(1, PAIRS):
                    _emit_kq_piece(0, tsl0, xk0, pair, 0)
                    _emit_kq_piece(0, tsl0, xq0, pair, 1)
                for tl in range(KPB):
                    _emit_v_one(0, xv0, tl)
                for qb in range(QB):
                    if qb + 2 < QB:
                        prefetched[qb + 2] = emit_proj_dma(qb + 2)
                    fillers = []
                    if qb + 1 < QB:
                        fillers += proj_pieces(qb + 1, prefetched.pop(qb + 1))
                    if qb == QB - 1:
                        for w in range(QB - 1):
                            fillers += wo_pieces(w)
                    emit_attn(qb, fillers)
                emit_wo(QB - 1, use_act=True)
            else:
                emit_wo_dma()
                for ch in range(QB):
                    emit_proj(ch, (xk0, xq0, xv0) if ch == 0 else None)
                for qb in range(QB):
                    emit_attn(qb)
                    emit_wo(qb)

            opool_cm.__exit__(None, None, None)
            fpool_cm.__exit__(None, None, None)
            pool_cm.__exit__(None, None, None)

    nc.compile()
    return nc


_PROGRAMS = {}


def _get_program(mode, seq=SEQ, d_model=D_MODEL, num_heads=NUM_HEADS):
    key = (mode, seq, d_model, num_heads)
    if key not in _PROGRAMS:
        _PROGRAMS[key] = build_program(seq, d_model, num_heads, mode)
    return _PROGRAMS[key]


def _detect_mode(mask, seq):
    m = np.asarray(mask)
    if (m != 0).all():
        return "dense"
    tril = np.tril(np.ones((seq, seq), np.int8))
    if np.array_equal((m != 0).astype(np.int8), tril):
        return "causal"
    return "mask"


def _swizzle_x(X, seq, d_model):
    # [seq, d_model] -> [P, QB, IN_T, QBS]: x[p, ch, it, t] = X[ch*QBS+t,
    # it*P+p], so each per-chunk DMA is contiguous per partition.
    QBS = min(512, seq)
    QB = seq // QBS
    IN_T = d_model // P
    return np.ascontiguousarray(
        X.reshape(QB, QBS, IN_T, P).transpose(3, 0, 2, 1)).astype(np.float16)


def _swizzle_w(Whalf, d_model, FL):
    # [FL, d_model] -> [P, IN_T, FL]: w[p, it, f] = Whalf[f, it*P+p]
    IN_T = d_model // P
    return np.ascontiguousarray(
        Whalf.reshape(FL, IN_T, P).transpose(2, 1, 0)).astype(np.float16)


def prep_inputs(Q, K, V, mask, Wq, bq, Wk, bk, Wv, bv, Wo, bo,
                num_heads=NUM_HEADS, mode=None):
    batch, seq, d_model = Q.shape
    HL = num_heads // 2
    FL = HL * (d_model // num_heads)
    FT = FL // P
    PAIRS = HL // 2
    if mode is None:
        mode = _detect_mode(mask, seq)
    maskt = None
    if mode == "mask":
        maskt = np.ascontiguousarray(
            (np.asarray(mask) != 0).astype(np.float16).T)
    in_maps = []
    for b in range(batch):
        xtq = _swizzle_x(np.asarray(Q[b], np.float32), seq, d_model)
        xtk = _swizzle_x(np.asarray(K[b], np.float32), seq, d_model)
        xtv = _swizzle_x(np.asarray(V[b], np.float32), seq, d_model)
        for half in range(2):
            fsl = slice(half * FL, (half + 1) * FL)
            # wot[p, ft, o] = Wo[o, fsl][ft*P+p] (lhsT rows of Wo^T)
            wot = np.ascontiguousarray(
                Wo[:, fsl].T.reshape(FT, P, d_model).transpose(1, 0, 2)
            ).astype(np.float16)
            im = {
                "xtq": xtq, "xtk": xtk, "xtv": xtv,
                "wqt": _swizzle_w(Wq[fsl, :], d_model, FL),
                "wkt": _swizzle_w(Wk[fsl, :], d_model, FL),
                "wvt": _swizzle_w(Wv[fsl, :], d_model, FL),
                "bqr": np.ascontiguousarray(
                    bq[fsl].reshape(PAIRS, P).T).astype(np.float32),
                "bkr": np.ascontiguousarray(
                    bk[fsl].reshape(PAIRS, P).T).astype(np.float32),
                "bvrow": bv[fsl].reshape(1, FL).astype(np.float16),
                "wot": wot,
            }
            if maskt is not None:
                im["maskt"] = maskt
            in_maps.append(im)
    return in_maps, mode


def _install_trace_hooks():
    """Provide antenv.axon_hooks (missing in this image) so that
    run_bass_kernel_spmd(trace=True) can capture NTFF profiles via the
    axon PJRT .so.  Bench-only; the graded path never enables tracing."""
    import contextlib
    import ctypes
    import types
    try:
        from antenv import axon_hooks  # noqa: F401
        return
    except ImportError:
        pass
    lib = ctypes.CDLL("/opt/axon/libaxon_pjrt.so")
    if not hasattr(lib, "axon_start_nrt_profile"):
        return
    lib.axon_start_nrt_profile.argtypes = [ctypes.POINTER(ctypes.c_int64),
                                           ctypes.c_size_t]
    lib.axon_start_nrt_profile.restype = ctypes.c_int64
    lib.axon_stop_nrt_profile.argtypes = [ctypes.c_char_p]
    lib.axon_stop_nrt_profile.restype = ctypes.c_int64

    @contextlib.contextmanager
    def _hook(output_dir, device_ids):
        import jax
        jax.devices()
        if device_ids:
            ids = (ctypes.c_int64 * len(device_ids))(*device_ids)
            rc = lib.axon_start_nrt_profile(ids, len(device_ids))
        else:
            rc = lib.axon_start_nrt_profile(None, 0)
        if rc != 0:
            raise RuntimeError(f"axon_start_nrt_profile rc={rc}")
        try:
            yield
        finally:
            n = lib.axon_stop_nrt_profile(str(output_dir).encode())
            print(f"profile: {n} file(s) written to {output_dir}", file=sys.stderr)

    mod = types.ModuleType("antenv.axon_hooks")
    mod.get_axon_ntff_profile_hook = lambda: _hook
    mod.set_axon_ntff_profile_hook = lambda h: None
    sys.modules["antenv.axon_hooks"] = mod
    import concourse.bass_utils as bu
    bu.upload_artifacts = lambda tmpdir: f"local:{tmpdir}"


def kernel(Q, K, V, mask, Wq, bq, Wk, bk, Wv, bv, Wo, bo):
    global LAST_EXEC_NS, LAST_RESULTS
    Q = np.asarray(Q); K = np.asarray(K); V = np.asarray(V)
    mask = np.asarray(mask)
    Wq = np.asarray(Wq, np.float32); bq = np.asarray(bq, np.float32)
    Wk = np.asarray(Wk, np.float32); bk = np.asarray(bk, np.float32)
    Wv = np.asarray(Wv, np.float32); bv = np.asarray(bv, np.float32)
    Wo = np.asarray(Wo, np.float32); bo = np.asarray(bo, np.float32)
    batch, seq, d_model = Q.shape

    in_maps, mode = prep_inputs(Q, K, V, mask, Wq, bq, Wk, bk, Wv, bv, Wo, bo)
    nc = _get_program(mode, seq, d_model, NUM_HEADS)

    trace = bool(os.environ.get("KBENCH_TRACE"))
    tmpdir = os.environ.get("KBENCH_TRACE_DIR") or None
    if trace:
        _install_trace_hooks()
    res = run_bass_kernel_spmd(nc, in_maps, list(range(N_CORES)), trace=trace,
                               tmpdir=tmpdir)
    LAST_EXEC_NS = res.exec_time_ns
    LAST_RESULTS = res
    out = np.empty((batch, seq, d_model), np.float32)
    for b in range(batch):
        out[b] = (res.results[2 * b]["out"].astype(np.float32)
                  + res.results[2 * b + 1]["out"].astype(np.float32) + bo)
    return out


# revision 39
# speedup vs baseline: 1.0163x; 1.0138x over previous
"""Multi-head attention TRN2 kernel (8 NeuronCores).

Sharding: core (2b + h2) handles batch b (of 4) and head-half h2 (8 of 16
heads).  Each core projects its batch's Q/K/V through its 512-column slice
of Wq/Wk/Wv, runs causal flash-attention for its 8 heads, and computes a
partial output projection through its 512 rows of Wo^T.  The two partial
outputs per batch are summed on the host (the "all-reduce after W_o"),
plus the output bias.

All matmuls run in fp16 with fp32 PSUM accumulation.  Scores are computed
transposed (S^T[kj, qi] = kT.T @ qT) so the softmax sum comes for free from
a ones-column appended to V (padded to 128 columns so LDWEIGHTS gets fast
weight load), and the attention output lands f-major, which is exactly the
lhsT layout the Wo matmul needs.  Normalisation (divide by the softmax sum)
is a DVE fast-reciprocal + GPSIMD partition-broadcast + DVE multiply.

Causal handling: on the 4 diagonal key-tiles of each 512-token query block
only the q-columns at or past the causal boundary are computed (widths
512/384/256/128 for scores, exp and AV), and the remaining triangular edge
is zeroed with a single shared 128x128 0/1 strip on the DVE.  Off-diagonal
upper tiles are skipped entirely.

Scheduling: the AV matmul for score-chunk i is emitted after the score
matmuls of chunk i+1, so its wait on the ACT exp resolves early and the PE
weight loads stay hidden.  Attention for query-block qb interleaves the
projections for chunk qb+1 as PE fillers (the exp-paced attention leaves
the PE underused), and the ACT-heaviest final block absorbs all deferred
output projections; only the last block's Wo remains as a tail.
"""

import os
import sys
import time

sys.path.insert(0, "/opt/trn_rl_repo")

import numpy as np

import concourse.bass as bass
import concourse.mybir as mybir
import concourse.tile as tile
from concourse import bacc
from concourse.bass_utils import run_bass_kernel_spmd
from concourse.masks import make_identity

F16 = mybir.dt.float16
F32 = mybir.dt.float32
P = 128

# Problem constants (full size).
D_MODEL = 1024
NUM_HEADS = 16
DK = D_MODEL // NUM_HEADS  # 64
BATCH = 4
SEQ = 2048
N_CORES = 8

LAST_EXEC_NS = None
LAST_RESULTS = None


def build_program(seq=SEQ, d_model=D_MODEL, num_heads=NUM_HEADS, mode="causal"):
    """Build the per-core Bass program.  Uniform across cores (SPMD).

    mode: "causal" (tril mask, block-skip + partial-width diagonal),
          "dense"  (no mask),
          "mask"   (arbitrary 0/1 mask, multiplicative, streamed from DRAM).
    """
    assert d_model % 256 == 0 and seq % P == 0
    HL = num_heads // 2              # local heads per core
    PAIRS = HL // 2                  # head-pairs (128 partitions each)
    FL = HL * DK                     # local features (columns of W slices)
    IN_T = d_model // P              # input-dim tiles
    FT = FL // P                     # local f tiles
    TT = seq // P                    # token tiles
    QBS = min(512, seq)              # qi block size
    QB = seq // QBS                  # qi blocks
    KJ = seq // P                    # key tiles
    KPB = QBS // P                   # key tiles per qi block (diag width)
    OFC = (d_model + 511) // 512     # output-feature chunks
    OFS = min(512, d_model)
    assert PAIRS >= 1 and FT >= 1 and QB >= 1

    nc = bacc.Bacc()
    # x and weights are pre-swizzled on the host to the SBUF layout so every
    # DMA moves 4-8 KiB contiguous per partition (strided 1KiB-row gathers
    # cap the DMA engines well below peak).
    xtq = nc.declare_dram_parameter("xtq", [P, QB, IN_T, QBS], F16,
                                    isOutput=False)
    xtk = nc.declare_dram_parameter("xtk", [P, QB, IN_T, QBS], F16,
                                    isOutput=False)
    xtv = nc.declare_dram_parameter("xtv", [P, QB, IN_T, QBS], F16,
                                    isOutput=False)
    wqt = nc.declare_dram_parameter("wqt", [P, IN_T, FL], F16, isOutput=False)
    wkt = nc.declare_dram_parameter("wkt", [P, IN_T, FL], F16, isOutput=False)
    wvt = nc.declare_dram_parameter("wvt", [P, IN_T, FL], F16, isOutput=False)
    bqr = nc.declare_dram_parameter("bqr", [P, PAIRS], F32, isOutput=False)
    bkr = nc.declare_dram_parameter("bkr", [P, PAIRS], F32, isOutput=False)
    bvrow = nc.declare_dram_parameter("bvrow", [1, FL], F16, isOutput=False)
    wot = nc.declare_dram_parameter("wot", [P, FT, d_model], F16,
                                    isOutput=False)
    if mode == "mask":
        maskt = nc.declare_dram_parameter("maskt", [seq, seq], F16, isOutput=False)
    out = nc.declare_dram_parameter("out", [seq, d_model], F16, isOutput=True)

    AF = mybir.ActivationFunctionType

    with tile.TileContext(nc) as tc:
        with (
            tc.tile_pool(name="const", bufs=1) as cpool,
            tc.tile_pool(name="big", bufs=1) as big,
            tc.tile_pool(name="xs", bufs=2) as xs,
            tc.tile_pool(name="es", bufs=6) as esp,
            tc.tile_pool(name="ep", bufs=2) as epi,
            tc.tile_pool(name="osb", bufs=3) as osb,
        ):
            # ---- constants.  Biases first (tiny, gate DVE adds), then the
            # weight/x halves interleaved in exactly PE consumption order so
            # the first K-projection can start as early as possible.  wo is
            # deferred past the chunk-1 prefetch (only needed by Wo fillers).
            bqr_sb = cpool.tile([P, PAIRS], F32)
            nc.sync.dma_start(bqr_sb[:], bqr[:, :])
            bkr_sb = cpool.tile([P, PAIRS], F32)
            nc.sync.dma_start(bkr_sb[:], bkr[:, :])
            bv_sb = cpool.tile([1, FL], F16)
            nc.sync.dma_start(bv_sb[:], bvrow[:, :])
            wk_sb = cpool.tile([P, IN_T, FL], F16)
            wq_sb = cpool.tile([P, IN_T, FL], F16)
            wv_sb = cpool.tile([P, IN_T, FL], F16)
            H_IT = IN_T // 2
            xk0 = xs.tile([P, IN_T, QBS], F16, tag="xk", name="xk_0")
            xq0 = xs.tile([P, IN_T, QBS], F16, tag="xq", name="xq_0")
            xv0 = xs.tile([P, IN_T, QBS], F16, tag="xv", name="xv_0")
            # split the setup burst across both HWDGE queues (SP carries K
            # then V, ACT carries Q) — one queue caps well below HBM
            # bandwidth and the start is DMA-bound; ACT is idle until the
            # first exp.
            for eng, w_sb, _wr, x_t, _xr in (
                    (nc.sync, wk_sb, wkt, xk0, xtk),
                    (nc.scalar, wq_sb, wqt, xq0, xtq),
                    (nc.sync, wv_sb, wvt, xv0, xtv)):
                eng.dma_start(w_sb[:, 0:H_IT, :], _wr[:, 0:H_IT, :])
                eng.dma_start(x_t[:, 0:H_IT, :], _xr[:, 0, 0:H_IT, :])
                eng.dma_start(w_sb[:, H_IT:IN_T, :], _wr[:, H_IT:IN_T, :])
                eng.dma_start(x_t[:, H_IT:IN_T, :], _xr[:, 0, H_IT:IN_T, :])
            wo_sb = cpool.tile([P, FT, d_model], F16)

            def emit_wo_dma():
                nc.sync.dma_start(wo_sb[:], wot[:, :, :])

            ones1 = cpool.tile([1, P], F16)
            nc.gpsimd.memset(ones1[:], 1.0)
            bvb = cpool.tile([P, FL], F16)
            nc.gpsimd.partition_broadcast(bvb[:], bv_sb[0:1, :])
            # shared 128x128 lower-triangular 0/1 strip: keep t >= k
            tri = cpool.tile([P, P], F16)
            nc.gpsimd.memset(tri[:], 1.0)
            nc.gpsimd.affine_select(
                out=tri[:], in_=tri[:],
                compare_op=mybir.AluOpType.is_ge,
                fill=0.0, base=0,
                pattern=[[1, P]], channel_multiplier=-1)

            # ---- persistent activations ----
            VW = DK + 2  # V columns + softmax-ones + pad (small LDWEIGHTS)
            qT_sb = big.tile([P, PAIRS, seq], F16)   # [2-head f, pair, tok]
            kT_sb = big.tile([P, PAIRS, seq], F16)
            v_sb = big.tile([P, TT, HL, VW], F16)  # [tok_in_tile, kj, h, d|1]
            oT_sb = big.tile([P, FT, seq], F16)      # attention out, f-major

            nc.gpsimd.memset(v_sb[:], 0.0)
            nc.gpsimd.memset(v_sb[:, :, :, DK:DK + 1], 1.0)

            # warm the ACT exp table early (one-time ~2.7us load)
            es_warm = esp.tile([1, 8], F16, tag="warm")
            nc.scalar.activation(es_warm[:], ones1[0:1, 0:8], AF.Exp, scale=1.0)

            # PSUM: scores get their own 2-deep ring of [128,2,512] tiles
            # (tag "s", 4 banks) and the projection/Wo filler pieces a
            # separate 2-deep ring of single-bank [128,512] tiles (tag "f",
            # 2 banks), so a filler's PSUM reuse never blocks the score
            # pipeline (and vice versa); + attention out tag "o" (2 banks).
            pool_cm = tc.tile_pool(name="pmm", bufs=2, space="PSUM")
            pmm = pool_cm.__enter__()
            fpool_cm = tc.tile_pool(name="pf", bufs=2, space="PSUM")
            pfil = fpool_cm.__enter__()
            opool_cm = tc.tile_pool(name="po", bufs=2, space="PSUM")
            pop = opool_cm.__enter__()

            def emit_proj_dma(ch):
                    xk_t = xs.tile([P, IN_T, QBS], F16, tag="xk",
                                   name=f"xk_{ch}")
                    nc.sync.dma_start(xk_t[:], xtk[:, ch, :, :])
                    xq_t = xs.tile([P, IN_T, QBS], F16, tag="xq",
                                   name=f"xq_{ch}")
                    nc.sync.dma_start(xq_t[:], xtq[:, ch, :, :])
                    xv_t = xs.tile([P, IN_T, QBS], F16, tag="xv",
                                   name=f"xv_{ch}")
                    nc.sync.dma_start(xv_t[:], xtv[:, ch, :, :])
                    return xk_t, xq_t, xv_t

            def proj_pieces(ch, tiles=None):
                    tsl = slice(ch * QBS, (ch + 1) * QBS)
                    xk_t, xq_t, xv_t = tiles if tiles else emit_proj_dma(ch)
                    pieces = []
                    def kq_piece(pair, which):
                        def go():
                            _emit_kq_piece(ch, tsl, xk_t if which == 0 else xq_t,
                                           pair, which)
                        return go
                    def v_piece(tl):
                        def go():
                            _emit_v_one(ch, xv_t, tl)
                        return go
                    for pair in range(PAIRS):
                        pieces.append(kq_piece(pair, 0))
                        pieces.append(kq_piece(pair, 1))
                    for tl in range(KPB):
                        pieces.append(v_piece(tl))
                    return pieces

            def emit_proj(ch, tiles=None):
                    for piece in proj_pieces(ch, tiles):
                        piece()

            def _emit_kq_piece(ch, tsl, x_t, pair, which):
                        fsl = slice(pair * P, (pair + 1) * P)
                        w_sb = wk_sb if which == 0 else wq_sb
                        dst = kT_sb if which == 0 else qT_sb
                        br = bkr_sb if which == 0 else bqr_sb
                        ps = pfil.tile([P, QBS], F32, tag="f",
                                       name=f"kq_{ch}_{pair}_{which}")
                        for it in range(IN_T):
                            nc.tensor.matmul(ps[:], w_sb[:, it, fsl],
                                             x_t[:, it, :],
                                             start=(it == 0), stop=(it == IN_T - 1))
                        nc.vector.tensor_scalar_add(dst[:, pair, tsl], ps[:],
                                                    br[:, pair:pair + 1])

            def _emit_v_one(ch, xv_t, tl):
                        tt = ch * KPB + tl
                        v_ps = pfil.tile([P, QBS], F32, tag="f",
                                         name=f"v_{ch}_{tl}")
                        for it in range(IN_T):
                            nc.tensor.matmul(
                                v_ps[:, 0:FL],
                                xv_t[:, it, tl * P:(tl + 1) * P],
                                wv_sb[:, it, :],
                                start=(it == 0), stop=(it == IN_T - 1))
                        nc.vector.tensor_tensor(
                            v_sb[:, tt, :, 0:DK],
                            v_ps[:, 0:FL].rearrange("p (h d) -> p h d", h=HL),
                            bvb[:].rearrange("p (h d) -> p h d", h=HL),
                            mybir.AluOpType.add)

            # ---- attention for qi block qb, all local heads ----
            # Chunk specs: list of [(kj, q_off), ...] (1-2 entries).  On the
            # causal diagonal q_off restricts scores/exp/AV to columns at or
            # past the causal boundary of that key tile.
            def attn_chunk_specs(qb):
                specs = []
                if mode == "causal":
                    d = qb * KPB
                    for j0 in range(0, KPB, 2):
                        specs.append([(d + j0, j0 * P), (d + j0 + 1, (j0 + 1) * P)])
                    for kj0 in range(0, qb * KPB, 2):
                        specs.append([(kj0, 0), (kj0 + 1, 0)])
                else:
                    for kj0 in range(0, KJ, 2):
                        if kj0 + 1 < KJ:
                            specs.append([(kj0, 0), (kj0 + 1, 0)])
                        else:
                            specs.append([(kj0, 0)])
                return specs

            def emit_attn(qb, fillers=()):
                fillers = list(fillers)
                qsl = slice(qb * QBS, (qb + 1) * QBS)
                specs = attn_chunk_specs(qb)
                n_chunks = len(specs)
                # The softmax-sum normalisation of head h is emitted one head
                # late (during head h+1's first chunks): its DVE chain would
                # otherwise head-of-line-block the next head's causal-edge
                # multiplies in the strict-FIFO DVE queue, stalling the PE.
                pending_norm = [None]

                def emit_norm():
                    if pending_norm[0] is None:
                        return
                    hh, o_prev = pending_norm[0]
                    pending_norm[0] = None
                    pair_p = hh // 2
                    po_p = (hh % 2) * DK
                    # (the sum is copied to SBUF first: the custom-DVE fast
                    # reciprocal misbehaves on PSUM operands on hardware)
                    srow = epi.tile([1, QBS], F32, tag="srow")
                    nc.vector.tensor_copy(srow[:], o_prev[DK:DK + 1, :])
                    recip_row = epi.tile([1, QBS], F32, tag="recip_row")
                    nc.vector.reciprocal_approx_fast(recip_row[:], srow[:])
                    recipb = epi.tile([DK, QBS], F32, tag="recipb")
                    nc.gpsimd.partition_broadcast(recipb[:], recip_row[0:1, :])
                    nc.vector.tensor_mul(oT_sb[po_p:po_p + DK, pair_p, qsl],
                                         o_prev[0:DK, :], recipb[:])

                for h in range(HL):
                    pair = h // 2
                    po = (h % 2) * DK
                    o_ps = pop.tile([P, QBS], F32, tag="o",
                                    name=f"o_{qb}_{h}")

                    def emit_scores(ci):
                        spec = specs[ci]
                        s_ps = pmm.tile([P, 2, QBS], F32, tag="s",
                                        name=f"s_{qb}_{h}_{ci}")
                        es = esp.tile([P, 2, QBS], F16, tag="es",
                                      name=f"es_{qb}_{h}_{ci}")
                        # scores are computed over the chunk's widest column
                        # range so a single fused exp can cover the whole
                        # chunk (fewer ACT instructions — exp paces the busy
                        # attention phases); AV still skips the columns the
                        # causal mask fully excludes per key tile.
                        off0 = min(off for _, off in spec)
                        for i, (kj, off) in enumerate(spec):
                            nc.tensor.matmul(
                                s_ps[:, i, off0:QBS],
                                kT_sb[po:po + DK, pair, kj * P:(kj + 1) * P],
                                qT_sb[po:po + DK, pair,
                                      qb * QBS + off0:(qb + 1) * QBS],
                                start=True, stop=True)
                        n = len(spec)
                        nc.scalar.activation(es[:, :n, off0:QBS],
                                             s_ps[:, :n, off0:QBS],
                                             AF.Exp, scale=0.125)
                        for i, (kj, off) in enumerate(spec):
                            if mode == "causal" and kj // KPB == qb:
                                j = kj % KPB
                                csl = slice(j * P, (j + 1) * P)
                                nc.vector.tensor_mul(es[:, i, csl],
                                                     es[:, i, csl], tri[:])
                            elif mode == "mask":
                                m_t = esp.tile([P, QBS], F16, tag="mt")
                                nc.sync.dma_start(
                                    m_t[:], maskt[kj * P:(kj + 1) * P, qsl])
                                nc.vector.tensor_mul(es[:, i, :], es[:, i, :],
                                                     m_t[:])
                        return es

                    def emit_av(ci, es):
                        spec = specs[ci]
                        for i, (kj, off) in enumerate(spec):
                            nc.tensor.matmul(
                                o_ps[0:VW, off:QBS], v_sb[:, kj, h, :],
                                es[:, i, off:QBS],
                                start=(ci == 0 and i == 0),
                                stop=(ci == n_chunks - 1 and i == len(spec) - 1))

                    # software pipeline: AV for chunk i after scores of i+1,
                    # so the exp wait resolves while the PE streams scores.
                    # Fillers go between the scores and the first AV: their PE
                    # work covers the latency of the DVE queue (causal-edge
                    # multiplies, delayed norm) that the first AV waits on.
                    quota = -(-len(fillers) // (HL - h))  # ceil: spread evenly
                    prev_es = emit_scores(0)
                    if n_chunks == 1:
                        emit_norm()
                        while quota > 0 and fillers:
                            fillers.pop(0)()
                            quota -= 1
                    for ci in range(1, n_chunks):
                        es = emit_scores(ci)
                        if ci == 1:
                            emit_norm()
                        if quota > 0 and fillers:
                            fillers.pop(0)()
                            quota -= 1
                        emit_av(ci - 1, prev_es)
                        prev_es = es
                    emit_av(n_chunks - 1, prev_es)
                    while quota > 0 and fillers:
                        fillers.pop(0)()
                        quota -= 1
                    pending_norm[0] = (h, o_ps)
                emit_norm()
                for f in fillers:
                    f()

            # ---- output projection for one token chunk ----
            def wo_pieces(qb, use_act=False):
                def tt_piece(tl, ofc):
                    def go():
                        _emit_wo_tt(qb, tl, ofc, use_act and ofc == 1)
                    return go
                return [tt_piece(tl, ofc)
                        for tl in range(KPB) for ofc in range(OFC)]

            def emit_wo(qb, use_act=False):
                for piece in wo_pieces(qb, use_act):
                    piece()

            def _emit_wo_tt(qb, tl, ofc, act_copy=False):
                    tt = qb * KPB + tl
                    osl = slice(ofc * OFS, (ofc + 1) * OFS)
                    w_ps = pfil.tile([P, QBS], F32, tag="f",
                                     name=f"w_{tt}_{ofc}")
                    for ft in range(FT):
                        nc.tensor.matmul(w_ps[:, 0:OFS],
                                         oT_sb[:, ft, tt * P:(tt + 1) * P],
                                         wo_sb[:, ft, osl],
                                         start=(ft == 0), stop=(ft == FT - 1))
                    o_out = osb.tile([P, OFS], F16, tag="oo")
                    if act_copy:
                        # tail only: ACT is idle there, halves the PSUM
                        # evacuation latency (Copy shares the Exp table)
                        nc.scalar.copy(o_out[:], w_ps[:, 0:OFS])
                    else:
                        nc.vector.tensor_copy(o_out[:], w_ps[:, 0:OFS])
                    nc.sync.dma_start(out[tt * P:(tt + 1) * P, osl], o_out[:])

            if mode == "causal":
                # x-chunk DMA runs one full attention phase ahead of the
                # projection fillers that consume it (chunk 1 right behind
                # the setup DMAs), so the PE stream never blocks on it.
                prefetched = {}
                if QB > 1:
                    prefetched[1] = emit_proj_dma(1)
                emit_wo_dma()
                emit_proj(0, (xk0, xq0, xv0))
                for qb in range(QB):
                    if qb + 2 < QB:
                        prefetched[qb + 2] = emit_proj_dma(qb + 2)
                    fillers = []
                    if qb + 1 < QB:
                        fillers += proj_pieces(qb + 1, prefetched.pop(qb + 1))
                    if qb == QB - 1:
                        for w in range(QB - 1):
                            fillers += wo_pieces(w)
                    emit_attn(qb, fillers)
                emit_wo(QB - 1, use_act=True)
            else:
                emit_wo_dma()
                for ch in range(QB):
                    emit_proj(ch, (xk0, xq0, xv0) if ch == 0 else None)
                for qb in range(QB):
                    emit_attn(qb)
                    emit_wo(qb)

            opool_cm.__exit__(None, None, None)
            fpool_cm.__exit__(None, None, None)
            pool_cm.__exit__(None, None, None)

    nc.compile()
    return nc


_PROGRAMS = {}


def _get_program(mode, seq=SEQ, d_model=D_MODEL, num_heads=NUM_HEADS):
    key = (mode, seq, d_model, num_heads)
    if key not in _PROGRAMS:
        _PROGRAMS[key] = build_program(seq, d_model, num_heads, mode)
    return _PROGRAMS[key]


def _detect_mode(mask, seq):
    m = np.asarray(mask)
    if (m != 0).all():
        return "dense"
    tril = np.tril(np.ones((seq, seq), np.int8))
    if np.array_equal((m != 0).astype(np.int8), tril):
        return "causal"
    return "mask"


def _swizzle_x(X, seq, d_model):
    # [seq, d_model] -> [P, QB, IN_T, QBS]: x[p, ch, it, t] = X[ch*QBS+t,
    # it*P+p], so each per-chunk DMA is contiguous per partition.
    QBS = min(512, seq)
    QB = seq // QBS
    IN_T = d_model // P
    return np.ascontiguousarray(
        X.reshape(QB, QBS, IN_T, P).transpose(3, 0, 2, 1)).astype(np.float16)


def _swizzle_w(Whalf, d_model, FL):
    # [FL, d_model] -> [P, IN_T, FL]: w[p, it, f] = Whalf[f, it*P+p]
    IN_T = d_model // P
    return np.ascontiguousarray(
        Whalf.reshape(FL, IN_T, P).transpose(2, 1, 0)).astype(np.float16)


def prep_inputs(Q, K, V, mask, Wq, bq, Wk, bk, Wv, bv, Wo, bo,
                num_heads=NUM_HEADS, mode=None):
    batch, seq, d_model = Q.shape
    HL = num_heads // 2
    FL = HL * (d_model // num_heads)
    FT = FL // P
    PAIRS = HL // 2
    if mode is None:
        mode = _detect_mode(mask, seq)
    maskt = None
    if mode == "mask":
        maskt = np.ascontiguousarray(
            (np.asarray(mask) != 0).astype(np.float16).T)
    in_maps = []
    for b in range(batch):
        xtq = _swizzle_x(np.asarray(Q[b], np.float32), seq, d_model)
        xtk = _swizzle_x(np.asarray(K[b], np.float32), seq, d_model)
        xtv = _swizzle_x(np.asarray(V[b], np.float32), seq, d_model)
        for half in range(2):
            fsl = slice(half * FL, (half + 1) * FL)
            # wot[p, ft, o] = Wo[o, fsl][ft*P+p] (lhsT rows of Wo^T)
            wot = np.ascontiguousarray(
                Wo[:, fsl].T.reshape(FT, P, d_model).transpose(1, 0, 2)
            ).astype(np.float16)
            im = {
                "xtq": xtq, "xtk": xtk, "xtv": xtv,
                "wqt": _swizzle_w(Wq[fsl, :], d_model, FL),
                "wkt": _swizzle_w(Wk[fsl, :], d_model, FL),
                "wvt": _swizzle_w(Wv[fsl, :], d_model, FL),
                "bqr": np.ascontiguousarray(
                    bq[fsl].reshape(PAIRS, P).T).astype(np.float32),
                "bkr": np.ascontiguousarray(
                    bk[fsl].reshape(PAIRS, P).T).astype(np.float32),
                "bvrow": bv[fsl].reshape(1, FL).astype(np.float16),
                "wot": wot,
            }
            if maskt is not None:
                im["maskt"] = maskt
            in_maps.append(im)
    return in_maps, mode


def _install_trace_hooks():
    """Provide antenv.axon_hooks (missing in this image) so that
    run_bass_kernel_spmd(trace=True) can capture NTFF profiles via the
    axon PJRT .so.  Bench-only; the graded path never enables tracing."""
    import contextlib
    import ctypes
    import types
    try:
        from antenv import axon_hooks  # noqa: F401
        return
    except ImportError:
        pass
    lib = ctypes.CDLL("/opt/axon/libaxon_pjrt.so")
    if not hasattr(lib, "axon_start_nrt_profile"):
        return
    lib.axon_start_nrt_profile.argtypes = [ctypes.POINTER(ctypes.c_int64),
                                           ctypes.c_size_t]
    lib.axon_start_nrt_profile.restype = ctypes.c_int64
    lib.axon_stop_nrt_profile.argtypes = [ctypes.c_char_p]
    lib.axon_stop_nrt_profile.restype = ctypes.c_int64

    @contextlib.contextmanager
    def _hook(output_dir, device_ids):
        import jax
        jax.devices()
        if device_ids:
            ids = (ctypes.c_int64 * len(device_ids))(*device_ids)
            rc = lib.axon_start_nrt_profile(ids, len(device_ids))
        else:
            rc = lib.axon_start_nrt_profile(None, 0)
        if rc != 0:
            raise RuntimeError(f"axon_start_nrt_profile rc={rc}")
        try:
            yield
        finally:
            n = lib.axon_stop_nrt_profile(str(output_dir).encode())
            print(f"profile: {n} file(s) written to {output_dir}", file=sys.stderr)

    mod = types.ModuleType("antenv.axon_hooks")
    mod.get_axon_ntff_profile_hook = lambda: _hook
    mod.set_axon_ntff_profile_hook = lambda h: None
    sys.modules["antenv.axon_hooks"] = mod
    import concourse.bass_utils as bu
    bu.upload_artifacts = lambda tmpdir: f"local:{tmpdir}"


def kernel(Q, K, V, mask, Wq, bq, Wk, bk, Wv, bv, Wo, bo):
    global LAST_EXEC_NS, LAST_RESULTS
    Q = np.asarray(Q); K = np.asarray(K); V = np.asarray(V)
    mask = np.asarray(mask)
    Wq = np.asarray(Wq, np.float32); bq = np.asarray(bq, np.float32)
    Wk = np.asarray(Wk, np.float32); bk = np.asarray(bk, np.float32)
    Wv = np.asarray(Wv, np.float32); bv = np.asarray(bv, np.float32)
    Wo = np.asarray(Wo, np.float32); bo = np.asarray(bo, np.float32)
    batch, seq, d_model = Q.shape

    in_maps, mode = prep_inputs(Q, K, V, mask, Wq, bq, Wk, bk, Wv, bv, Wo, bo)
    nc = _get_program(mode, seq, d_model, NUM_HEADS)

    trace = bool(os.environ.get("KBENCH_TRACE"))
    tmpdir = os.environ.get("KBENCH_TRACE_DIR") or None
    if trace:
        _install_trace_hooks()
    res = run_bass_kernel_spmd(nc, in_maps, list(range(N_CORES)), trace=trace,
                               tmpdir=tmpdir)
    LAST_EXEC_NS = res.exec_time_ns
    LAST_RESULTS = res
    out = np.empty((batch, seq, d_model), np.float32)
    for b in range(batch):
        out[b] = (res.results[2 * b]["out"].astype(np.float32)
                  + res.results[2 * b + 1]["out"].astype(np.float32) + bo)
    return out
